# revision 1
# baseline (speedup 1.0000x reference)
"""Causal self-attention (B=4, T=2048, C=1024, H=16) on 8 TRN2 NeuronCores.

Sharding: 8 cores = 4 batches x 2 head-groups (Megatron tensor-parallel over
heads + data-parallel over batch). Each core computes, for its batch b and its
8 heads:
  stage 1: qkv projection (column-parallel slice of w_qkv)
           -> qT, kT stored [head_dim on partitions, T on free] (pair-packed:
              even head dims on partitions 0-63, odd head on 64-127)
           -> V stored [T on partitions, head-major dims on free], with an
              appended ones-column per head (computes softmax sums for free)
  stage 2: causal attention computed TRANSPOSED (S^T = K^T-tiles x Q-tiles,
           keys on PSUM partitions, queries on free dim):
           - K=64 contractions pair-packed via tile_position row groups
           - causal triangle applied by accumulating a -1e5 triangular mask
             into PSUM via a bf16 identity matmul (only diagonal j-tiles)
           - exp on the Scalar engine straight out of PSUM (no max-subtraction:
             logits are O(1) by construction, 1/sqrt(hd) folded into w_q)
           - PV matmul with ones-augmented V gives unnormalized y^T plus the
             softmax sums in PSUM row 64; normalize with reciprocal +
             partition_broadcast
  stage 3: output projection partial (row-parallel slice of w_proj)
Host: shards inputs, sums the two partial outputs per batch, adds b_proj.

All heavy matmuls run as float32r (TF32-like, full PE rate at N>=256; inputs
are fp32 bits reinterpreted via bitcast).
"""

import sys

if "/opt/trn_rl_repo" not in sys.path:
    sys.path.insert(0, "/opt/trn_rl_repo")

from contextlib import ExitStack

import numpy as np
import ml_dtypes

import concourse.bass as bass
import concourse.tile as tile
from concourse import bacc, mybir
from concourse.bass_utils import run_bass_kernel_spmd

F32 = mybir.dt.float32
F32R = mybir.dt.float32r
BF16 = mybir.dt.bfloat16
AF = mybir.ActivationFunctionType

B, T, C = 4, 2048, 1024
H, HD = 16, 64
NHL = 8          # heads per core (local)
NPAIR = 4        # head pairs per core
P = 128
TQ = 512         # query tile (free dim)
TJ = 128         # key tile (partitions)
NIT = T // TQ    # 4 query tiles
NTS = T // P     # 16 token sub-tiles
NCT = C // P     # 8 contraction tiles over C
NEG = -100000.0  # additive causal mask value


def build_kernel(trace_label=None):
    nc = bacc.Bacc("TRN2", target_bir_lowering=False)

    xt = nc.declare_dram_parameter("xt", [C, T], F32R, isOutput=False)
    wqk = nc.declare_dram_parameter("wqk", [P, NCT, 1024], F32R, isOutput=False)
    bqk = nc.declare_dram_parameter("bqk", [P, 8], F32, isOutput=False)
    wv = nc.declare_dram_parameter("wv", [P, NCT, 512], F32R, isOutput=False)
    bv = nc.declare_dram_parameter("bv", [1, 512], F32, isOutput=False)
    wp = nc.declare_dram_parameter("wp", [P, NPAIR, 1024], F32R, isOutput=False)
    tri = nc.declare_dram_parameter("tri", [P, P], BF16, isOutput=False)
    idn = nc.declare_dram_parameter("idn", [P, P], BF16, isOutput=False)
    out = nc.declare_dram_parameter("out", [T, C], F32, isOutput=True)

    with tile.TileContext(nc) as tc, ExitStack() as ctx:
        persist = ctx.enter_context(tc.tile_pool(name="persist", bufs=1))

        q_sb = persist.tile([P, NPAIR, T], F32R)
        k_sb = persist.tile([P, NPAIR, T], F32R)
        v_sb = persist.tile([P, NTS, NHL, HD + 1], F32R)
        bqk_sb = persist.tile([P, 8], F32)
        bv_sb = persist.tile([P, 512], F32)
        tri_sb = persist.tile([P, P], BF16)
        idn_sb = persist.tile([P, P], BF16)
        wp_sb = persist.tile([P, NPAIR, 1024], F32R)

        nc.sync.dma_start(bqk_sb, bqk[:])
        nc.sync.dma_start(tri_sb, tri[:])
        nc.sync.dma_start(idn_sb, idn[:])
        # materialize bias-v broadcast across partitions
        nc.sync.dma_start(bv_sb[0:1, :], bv[:])
        nc.gpsimd.partition_broadcast(bv_sb[:, :], bv_sb[0:1, :])
        # ones columns of the augmented V
        nc.vector.memset(v_sb[:, :, :, HD : HD + 1].bitcast(F32), 1.0)

        # ---------------- stage 1: qkv projection ----------------
        with (
            tc.tile_pool(name="s1w", bufs=1) as s1w,
            tc.tile_pool(name="s1x", bufs=12) as s1x,
            tc.tile_pool(name="s1ps", bufs=3, space="PSUM") as s1ps,
        ):
            wqk_sb = s1w.tile([P, NCT, 1024], F32R)
            wv_sb = s1w.tile([P, NCT, 512], F32R)

            for t in range(NIT):
                t0 = t * TQ
                xc = []
                for c in range(NCT):
                    xi = s1x.tile([P, TQ], F32R, tag="xc")
                    nc.sync.dma_start(xi, xt[c * P : (c + 1) * P, t0 : t0 + TQ])
                    xc.append(xi)
                    if t == 0:
                        # stream weight chunks interleaved with the first x
                        # tiles so the c-accumulation paces with DMA arrival
                        # instead of stalling ~25us on monolithic loads
                        nc.sync.dma_start(wqk_sb[:, c, :], wqk[:, c, :])
                        nc.sync.dma_start(wv_sb[:, c, :], wv[:, c, :])
                # q (m 0-3) and k (m 4-7) blocks: out [f-part, t-free]
                for m in range(8):
                    ps = s1ps.tile([P, TQ], F32, tag="ps")
                    for c in range(NCT):
                        nc.tensor.matmul(
                            ps,
                            wqk_sb[:, c, m * P : (m + 1) * P],
                            xc[c],
                            start=(c == 0),
                            stop=(c == NCT - 1),
                        )
                    dst = q_sb if m < 4 else k_sb
                    nc.vector.tensor_scalar_add(
                        dst[:, m % 4, t0 : t0 + TQ], ps, bqk_sb[:, m : m + 1]
                    )
                # v blocks: out [t-part, f-free(head-major)]
                for s in range(TQ // P):
                    ps = s1ps.tile([P, 512], F32, tag="ps")
                    for c in range(NCT):
                        nc.tensor.matmul(
                            ps,
                            xc[c][:, s * P : (s + 1) * P],
                            wv_sb[:, c, :],
                            start=(c == 0),
                            stop=(c == NCT - 1),
                        )
                    tsub = t * (TQ // P) + s
                    nc.vector.tensor_tensor(
                        v_sb[:, tsub, :, 0:HD],
                        ps.rearrange("p (h d) -> p h d", h=NHL),
                        bv_sb.rearrange("p (h d) -> p h d", h=NHL),
                        mybir.AluOpType.add,
                    )

        # ---------------- stage 2: causal attention ----------------
        # wp is only needed by stage 3 — load it here so it doesn't
        # compete with the stage-1 weight/x streams
        nc.sync.dma_start(wp_sb, wp[:])
        with (
            tc.tile_pool(name="s2att", bufs=4) as s2att,
            tc.tile_pool(name="s2n", bufs=3) as s2n,
            tc.tile_pool(name="qkps", bufs=2, space="PSUM") as qkps,
            tc.tile_pool(name="pvps", bufs=4, space="PSUM") as pvps,
        ):
            for a in range(NPAIR):
                for it in range(NIT):
                    i0 = it * TQ
                    njt = (i0 + TQ) // TJ
                    pv = [
                        pvps.tile([P, TQ], F32, tag="pv", name=f"pv0_{a}_{it}"),
                        pvps.tile([P, TQ], F32, tag="pv", name=f"pv1_{a}_{it}"),
                    ]
                    for jt in range(njt):
                        j0 = jt * TJ
                        d = j0 - i0
                        istart = max(d, 0)
                        nn = TQ - istart
                        # one 2-bank psum tile holds both heads' S^T blocks;
                        # a single fused exp call halves ACT instruction count
                        qk = qkps.tile([P, 2, TQ], F32, tag="qk")
                        for e in (0, 1):
                            nc.tensor.matmul(
                                qk[:, e, istart:TQ],
                                k_sb[64 * e : 64 * e + 64, a, j0 : j0 + TJ],
                                q_sb[64 * e : 64 * e + 64, a, i0 + istart : i0 + TQ],
                                start=True,
                                stop=(d < 0),
                                tile_position=(64 * e, 0),
                            )
                            if d >= 0:
                                nc.tensor.matmul(
                                    qk[:, e, istart : istart + TJ],
                                    idn_sb,
                                    tri_sb,
                                    start=False,
                                    stop=True,
                                    tile_position=(0, 0),
                                )
                        att = s2att.tile([P, 2, TQ], F32R, tag="att")
                        nc.scalar.activation(
                            att[:, :, 0:nn], qk[:, :, istart:TQ], AF.Exp
                        )
                        for e in (0, 1):
                            nc.tensor.matmul(
                                pv[e][0 : HD + 1, istart:TQ],
                                v_sb[:, jt, 2 * a + e, :],
                                att[:, e, 0:nn],
                                start=(jt == 0),
                                stop=(jt == njt - 1),
                            )
                    for e in (0, 1):
                        rt = s2n.tile([P, TQ], F32, tag="rt")
                        nc.vector.reciprocal(rt[HD : HD + 1, :], pv[e][HD : HD + 1, :])
                        rb = s2n.tile([P, TQ], F32, tag="rb")
                        nc.sync.dma_start(rb[0:1, :], rt[HD : HD + 1, :])
                        nc.gpsimd.partition_broadcast(rb[0:HD, :], rb[0:1, :])
                        if e == 0:
                            nc.vector.tensor_mul(
                                q_sb[0:HD, a, i0 : i0 + TQ], pv[e][0:HD, :], rb[0:HD, :]
                            )
                        else:
                            yt = s2n.tile([P, TQ], F32R, tag="yt")
                            nc.vector.tensor_mul(yt[0:HD, :], pv[e][0:HD, :], rb[0:HD, :])
                            nc.sync.dma_start(
                                q_sb[64:128, a, i0 : i0 + TQ], yt[0:HD, :]
                            )


        # ---------------- stage 3: output projection ----------------
        # normalized y^T was written into q_sb (reuse; q no longer needed)
        with (
            tc.tile_pool(name="s3o", bufs=4) as s3o,
            tc.tile_pool(name="s3ps", bufs=4, space="PSUM") as s3ps,
        ):
            for tt in range(NTS):
                for ot in range(2):
                    ps = s3ps.tile([P, 512], F32, tag="ops")
                    for a in range(NPAIR):
                        nc.tensor.matmul(
                            ps,
                            q_sb[:, a, tt * P : (tt + 1) * P],
                            wp_sb[:, a, ot * 512 : (ot + 1) * 512],
                            start=(a == 0),
                            stop=(a == NPAIR - 1),
                        )
                    ot_sb = s3o.tile([P, 512], F32, tag="osb")
                    nc.vector.tensor_copy(ot_sb, ps)
                    nc.sync.dma_start(
                        out[tt * P : (tt + 1) * P, ot * 512 : (ot + 1) * 512], ot_sb
                    )

    nc.compile()
    return nc


_NC_CACHE = None


def _get_nc():
    global _NC_CACHE
    if _NC_CACHE is None:
        _NC_CACHE = build_kernel()
    return _NC_CACHE


def _shard_inputs(x, w_qkv, b_qkv, w_proj):
    """Build the 8 per-core input maps. Core id = 2*batch + head_group."""
    tri_np = np.where(
        np.arange(P)[None, :] >= np.arange(P)[:, None], 0.0, NEG
    ).astype(ml_dtypes.bfloat16)
    idn_np = np.eye(P, dtype=ml_dtypes.bfloat16)

    in_maps = []
    for b in range(B):
        xt = np.ascontiguousarray(x[b].T)  # [C, T]
        for g in range(2):
            s = slice(g * 512, (g + 1) * 512)
            wqk_full = np.concatenate(
                [w_qkv[0:1024][s] / 8.0, w_qkv[1024:2048][s]], axis=0
            )  # [1024 f, 1024 c]
            wqk_arr = np.ascontiguousarray(
                wqk_full.T.reshape(NCT, P, 1024).transpose(1, 0, 2)
            )
            bqk_full = np.concatenate([b_qkv[0:1024][s] / 8.0, b_qkv[1024:2048][s]])
            bqk_arr = np.ascontiguousarray(bqk_full.reshape(8, P).T)
            wv_rows = w_qkv[2048:3072][s]  # [512 f, 1024 c]
            wv_arr = np.ascontiguousarray(
                wv_rows.T.reshape(NCT, P, 512).transpose(1, 0, 2)
            )
            bv_arr = np.ascontiguousarray(b_qkv[2048:3072][s][None, :])
            wp_rhs = w_proj[:, s].T  # [512 hd, 1024 o]
            wp_arr = np.ascontiguousarray(
                wp_rhs.reshape(NPAIR, P, 1024).transpose(1, 0, 2)
            )
            in_maps.append(
                {
                    "xt": xt,
                    "wqk": wqk_arr.astype(np.float32),
                    "bqk": bqk_arr.astype(np.float32),
                    "wv": wv_arr.astype(np.float32),
                    "bv": bv_arr.astype(np.float32),
                    "wp": wp_arr.astype(np.float32),
                    "tri": tri_np,
                    "idn": idn_np,
                }
            )
    return in_maps


def kernel(x, w_qkv, b_qkv, w_proj, b_proj, _trace=False, _trace_kwargs=None):
    x = np.asarray(x, dtype=np.float32)
    w_qkv = np.asarray(w_qkv, dtype=np.float32)
    b_qkv = np.asarray(b_qkv, dtype=np.float32)
    w_proj = np.asarray(w_proj, dtype=np.float32)
    b_proj = np.asarray(b_proj, dtype=np.float32)

    nc = _get_nc()
    in_maps = _shard_inputs(x, w_qkv, b_qkv, w_proj)
    res = run_bass_kernel_spmd(
        nc, in_maps, core_ids=list(range(8)), trace=_trace,
        **(_trace_kwargs or {}),
    )
    out = np.empty((B, T, C), np.float32)
    for b in range(B):
        out[b] = res.results[2 * b]["out"] + res.results[2 * b + 1]["out"] + b_proj
    if _trace:
        return out, res
    return out



# revision 7
# speedup vs baseline: 1.0682x; 1.0682x over previous
"""Causal self-attention (B=4, T=2048, C=1024, H=16) on 8 TRN2 NeuronCores.

Sharding: 8 cores = 4 batches x 2 head-groups (Megatron tensor-parallel over
heads + data-parallel over batch). Each core computes, for its batch b and its
8 heads, qkv projection -> causal attention -> its partial output projection.
Host sums the two partial outputs per batch and adds b_proj.

Schedule: one fused band loop. For band t the emission order is
  stage1(t=0 only, standalone) ... then per band:
    prefetch x(t+1) [SP DMA queue]
    attention(it=t) jt-stream, with stage1(t+1) and proj(t-1) matmul groups
    interleaved INTO the stream
so the tensor engine always has independent work while the Scalar engine's
exp (the attention critical resource: ~1.03us per 512-col j-tile vs ~850ns
of PE work) drains. Weight loads ride the second (Activation) HWDGE queue in
parallel with x loads on the SP queue.

dtypes: x/q/k/v/att/y bf16 (moving-operand bf16 keeps every attention matmul
at 1 cycle/row; fp32r would drop to 4 cyc/row below 256 free columns),
weights f32r, all accumulation f32 in PSUM. exp needs no max-subtraction:
logits are O(1) by construction, 1/sqrt(hd) folded into w_q on the host.
Causality at 128-column granularity; the in-tile triangle is applied by
accumulating a -1e5 triangular bf16 mask into PSUM via an identity matmul.
Softmax sums come free from an appended ones-column on V; normalization is
reciprocal + partition_broadcast applied to the unnormalized PV output.
"""

import sys

if "/opt/trn_rl_repo" not in sys.path:
    sys.path.insert(0, "/opt/trn_rl_repo")

from contextlib import ExitStack

import numpy as np
import ml_dtypes

import concourse.bass as bass
import concourse.tile as tile
from concourse import bacc, mybir
from concourse.bass_utils import run_bass_kernel_spmd

F32 = mybir.dt.float32
F32R = mybir.dt.float32r
BF16 = mybir.dt.bfloat16
AF = mybir.ActivationFunctionType

B, T, C = 4, 2048, 1024
H, HD = 16, 64
NHL = 8          # heads per core (local)
NPAIR = 4        # head pairs per core
P = 128
TQ = 512         # query tile (free dim)
TJ = 128         # key tile (partitions)
NIT = T // TQ    # 4 query tile bands
NTS = T // P     # 16 token sub-tiles
NCT = C // P     # 8 contraction tiles over C
NEG = -100000.0  # additive causal mask value


def build_kernel(trace_label=None):
    nc = bacc.Bacc("TRN2", target_bir_lowering=False)

    xt = nc.declare_dram_parameter("xt", [C, T], F32R, isOutput=False)
    wqk = nc.declare_dram_parameter("wqk", [P, 8, 1024], F32R, isOutput=False)
    bqk = nc.declare_dram_parameter("bqk", [P, 8], F32, isOutput=False)
    wv = nc.declare_dram_parameter("wv", [P, NCT, 512], F32R, isOutput=False)
    bv = nc.declare_dram_parameter("bv", [1, 512], F32, isOutput=False)
    wp = nc.declare_dram_parameter("wp", [P, NPAIR, 1024], BF16, isOutput=False)
    tri = nc.declare_dram_parameter("tri", [P, P], BF16, isOutput=False)
    idn = nc.declare_dram_parameter("idn", [P, P], BF16, isOutput=False)
    out = nc.declare_dram_parameter("out", [T, C], F32, isOutput=True)

    with tile.TileContext(nc) as tc, ExitStack() as ctx:
        persist = ctx.enter_context(tc.tile_pool(name="persist", bufs=1))

        q_sb = persist.tile([P, NPAIR, T], BF16)   # queries; later y^T (reuse)
        k_sb = persist.tile([P, NPAIR, T], BF16)
        v_sb = persist.tile([P, NTS, NHL, HD + 1], BF16)
        bqk_sb = persist.tile([P, 8], F32)
        bv_sb = persist.tile([P, 512], F32)
        tri_sb = persist.tile([P, P], BF16)
        idn_sb = persist.tile([P, P], BF16)
        wqk_sb = persist.tile([P, 8, 1024], F32R)
        wv_sb = persist.tile([P, NCT, 512], F32R)
        wp_sb = persist.tile([P, NPAIR, 1024], BF16)

        s1x = ctx.enter_context(tc.tile_pool(name="s1x", bufs=16))
        attp = ctx.enter_context(tc.tile_pool(name="attp", bufs=4))
        nrm = ctx.enter_context(tc.tile_pool(name="nrm", bufs=2))
        s3o = ctx.enter_context(tc.tile_pool(name="s3o", bufs=4))
        mmps = ctx.enter_context(tc.tile_pool(name="mmps", bufs=2, space="PSUM"))
        qkps = ctx.enter_context(tc.tile_pool(name="qkps", bufs=2, space="PSUM"))
        pvps = ctx.enter_context(tc.tile_pool(name="pvps", bufs=2, space="PSUM"))

        # setup constants on the Activation HWDGE queue; x rides the SP queue
        nc.scalar.dma_start(bqk_sb, bqk[:])
        nc.scalar.dma_start(tri_sb, tri[:])
        nc.scalar.dma_start(idn_sb, idn[:])
        nc.scalar.dma_start(bv_sb[0:1, :], bv[:])
        nc.gpsimd.partition_broadcast(bv_sb[:, :], bv_sb[0:1, :])
        nc.vector.memset(v_sb[:, :, :, HD : HD + 1], 1.0)

        xc_tiles = {}

        def load_x(t):
            for c in range(NCT):
                xi = s1x.tile([P, TQ], F32R, tag="xc", name=f"x_{t}_{c}")
                nc.sync.dma_start(xi, xt[c * P : (c + 1) * P, t * TQ : (t + 1) * TQ])
                xc_tiles[(t, c)] = xi

        load_x(0)
        # weights interleaved m/c-chunk-wise so the t=0 accumulation groups
        # pace with DMA arrival instead of stalling on a monolithic load
        for i in range(NCT):
            nc.scalar.dma_start(wqk_sb[:, i, :], wqk[:, i, :])
            nc.scalar.dma_start(wv_sb[:, i, :], wv[:, i, :])
        nc.scalar.dma_start(wp_sb, wp[:])

        def emit_s1_qk(t, m):
            # q (m 0-3) / k (m 4-7) feature block: out [f-part, t-free]
            ps = mmps.tile([P, TQ], F32, tag="mm", name=f"s1qk_{t}_{m}")
            for c in range(NCT):
                nc.tensor.matmul(
                    ps,
                    wqk_sb[:, m, c * P : (c + 1) * P],
                    xc_tiles[(t, c)],
                    start=(c == 0),
                    stop=(c == NCT - 1),
                )
            dst = q_sb if m < 4 else k_sb
            nc.vector.tensor_scalar_add(
                dst[:, m % 4, t * TQ : (t + 1) * TQ], ps, bqk_sb[:, m : m + 1]
            )

        def emit_s1_v(t, s):
            # v block: out [t-part, f-free(head-major)] + bias, ones col kept
            ps = mmps.tile([P, 512], F32, tag="mm", name=f"s1v_{t}_{s}")
            for c in range(NCT):
                nc.tensor.matmul(
                    ps,
                    xc_tiles[(t, c)][:, s * P : (s + 1) * P],
                    wv_sb[:, c, :],
                    start=(c == 0),
                    stop=(c == NCT - 1),
                )
            nc.vector.tensor_tensor(
                v_sb[:, t * 4 + s, :, 0:HD],
                ps.rearrange("p (h d) -> p h d", h=NHL),
                bv_sb.rearrange("p (h d) -> p h d", h=NHL),
                mybir.AluOpType.add,
            )

        def emit_proj(tt, ot):
            ps = mmps.tile([P, 512], F32, tag="mm", name=f"s3_{tt}_{ot}")
            for a in range(NPAIR):
                nc.tensor.matmul(
                    ps,
                    q_sb[:, a, tt * P : (tt + 1) * P],
                    wp_sb[:, a, ot * 512 : (ot + 1) * 512],
                    start=(a == 0),
                    stop=(a == NPAIR - 1),
                )
            ot_sb = s3o.tile([P, 512], F32, tag="osb", name=f"o_{tt}_{ot}")
            nc.vector.tensor_copy(ot_sb, ps)
            nc.sync.dma_start(
                out[tt * P : (tt + 1) * P, ot * 512 : (ot + 1) * 512], ot_sb
            )

        def normalize(a, it, pv):
            i0 = it * TQ
            for e in (0, 1):
                rt = nrm.tile([P, TQ], F32, tag="rt", name=f"rt_{a}_{it}_{e}")
                nc.vector.reciprocal(rt[HD : HD + 1, :], pv[e][HD : HD + 1, :])
                rb = nrm.tile([P, TQ], F32, tag="rb", name=f"rb_{a}_{it}_{e}")
                nc.scalar.dma_start(rb[0:1, :], rt[HD : HD + 1, :])
                nc.gpsimd.partition_broadcast(rb[0:HD, :], rb[0:1, :])
                if e == 0:
                    nc.vector.tensor_mul(
                        q_sb[0:HD, a, i0 : i0 + TQ], pv[e][0:HD, :], rb[0:HD, :]
                    )
                else:
                    # e=1 heads land on partitions 64-127: DVE is lane-locked,
                    # so go through an SBUF tile + DMA partition shift
                    yt = nrm.tile([P, TQ], BF16, tag="yt", name=f"yt_{a}_{it}")
                    nc.vector.tensor_mul(yt[0:HD, :], pv[e][0:HD, :], rb[0:HD, :])
                    nc.scalar.dma_start(q_sb[64:128, a, i0 : i0 + TQ], yt[0:HD, :])

        def emit_attn_band(it, extra):
            i0 = it * TQ
            njt = (i0 + TQ) // TJ
            total_jt = njt * NPAIR
            interval = max(1, total_jt // (len(extra) + 1)) if extra else 0
            jcount = 0
            for a in range(NPAIR):
                pv = [
                    pvps.tile([P, TQ], F32, tag="pv", name=f"pv{e}_{a}_{it}")
                    for e in (0, 1)
                ]
                prev = None
                for jt in range(njt):
                    j0 = jt * TJ
                    d = j0 - i0
                    istart = max(d, 0)
                    nn = TQ - istart
                    # one 2-bank psum tile holds both heads' S^T blocks;
                    # a single fused exp call halves ACT instruction count
                    qk = qkps.tile([P, 2, TQ], F32, tag="qk", name=f"qk_{a}_{it}_{jt}")
                    for e in (0, 1):
                        nc.tensor.matmul(
                            qk[:, e, istart:TQ],
                            k_sb[64 * e : 64 * e + 64, a, j0 : j0 + TJ],
                            q_sb[64 * e : 64 * e + 64, a, i0 + istart : i0 + TQ],
                            start=True,
                            stop=(d < 0),
                            tile_position=(64 * e, 0),
                        )
                        if d >= 0:
                            nc.tensor.matmul(
                                qk[:, e, istart : istart + TJ],
                                idn_sb,
                                tri_sb,
                                start=False,
                                stop=True,
                                tile_position=(0, 0),
                            )
                    att = attp.tile(
                        [P, 2, TQ], BF16, tag="att", name=f"att_{a}_{it}_{jt}"
                    )
                    nc.scalar.activation(att[:, :, 0:nn], qk[:, :, istart:TQ], AF.Exp)
                    if prev is not None:
                        pjt, patt, pnn, pistart = prev
                        for e in (0, 1):
                            nc.tensor.matmul(
                                pv[e][0 : HD + 1, pistart:TQ],
                                v_sb[:, pjt, 2 * a + e, :],
                                patt[:, e, 0:pnn],
                                start=(pjt == 0),
                                stop=(pjt == njt - 1),
                            )
                    prev = (jt, att, nn, istart)
                    jcount += 1
                    if extra and jcount % interval == 0:
                        extra.pop(0)()
                pjt, patt, pnn, pistart = prev
                for e in (0, 1):
                    nc.tensor.matmul(
                        pv[e][0 : HD + 1, pistart:TQ],
                        v_sb[:, pjt, 2 * a + e, :],
                        patt[:, e, 0:pnn],
                        start=(pjt == 0),
                        stop=(pjt == njt - 1),
                    )
                normalize(a, it, pv)
            while extra:
                extra.pop(0)()

        # band 0 stage 1 runs standalone (nothing to overlap with yet)
        for m in range(8):
            emit_s1_qk(0, m)
        for s in range(4):
            emit_s1_v(0, s)

        for it in range(NIT):
            t_next = it + 1
            extra = []
            if t_next < NIT:
                load_x(t_next)
                extra += [lambda m=m: emit_s1_qk(t_next, m) for m in range(8)]
                extra += [lambda s=s: emit_s1_v(t_next, s) for s in range(4)]
            if it >= 1:
                tb = it - 1
                extra += [
                    lambda tt=tt, ot=ot: emit_proj(tt, ot)
                    for tt in range(tb * 4, tb * 4 + 4)
                    for ot in range(2)
                ]
            emit_attn_band(it, extra)

        for tt in range(12, 16):
            for ot in range(2):
                emit_proj(tt, ot)

    nc.compile()
    return nc


_NC_CACHE = None


def _get_nc():
    global _NC_CACHE
    if _NC_CACHE is None:
        _NC_CACHE = build_kernel()
    return _NC_CACHE


def _shard_inputs(x, w_qkv, b_qkv, w_proj):
    """Build the 8 per-core input maps. Core id = 2*batch + head_group."""
    tri_np = np.where(
        np.arange(P)[None, :] >= np.arange(P)[:, None], 0.0, NEG
    ).astype(ml_dtypes.bfloat16)
    idn_np = np.eye(P, dtype=ml_dtypes.bfloat16)

    in_maps = []
    for b in range(B):
        xt = np.ascontiguousarray(x[b].T)  # [C, T]
        for g in range(2):
            s = slice(g * 512, (g + 1) * 512)
            wqk_full = np.concatenate(
                [w_qkv[0:1024][s] / 8.0, w_qkv[1024:2048][s]], axis=0
            )  # [1024 f, 1024 c]
            # m-major chunks: wqk_arr[p, m, c*128+j] = wqk_full[m*128+j, c*128+p]
            wqk_arr = np.ascontiguousarray(
                wqk_full.T.reshape(NCT, P, 8, P).transpose(1, 2, 0, 3).reshape(P, 8, 1024)
            )
            bqk_full = np.concatenate([b_qkv[0:1024][s] / 8.0, b_qkv[1024:2048][s]])
            bqk_arr = np.ascontiguousarray(bqk_full.reshape(8, P).T)
            wv_rows = w_qkv[2048:3072][s]  # [512 f, 1024 c]
            wv_arr = np.ascontiguousarray(
                wv_rows.T.reshape(NCT, P, 512).transpose(1, 0, 2)
            )
            bv_arr = np.ascontiguousarray(b_qkv[2048:3072][s][None, :])
            wp_rhs = w_proj[:, s].T  # [512 hd, 1024 o]
            wp_arr = np.ascontiguousarray(
                wp_rhs.reshape(NPAIR, P, 1024).transpose(1, 0, 2)
            )
            in_maps.append(
                {
                    "xt": xt,
                    "wqk": wqk_arr.astype(np.float32),
                    "bqk": bqk_arr.astype(np.float32),
                    "wv": wv_arr.astype(np.float32),
                    "bv": bv_arr.astype(np.float32),
                    "wp": wp_arr.astype(ml_dtypes.bfloat16),
                    "tri": tri_np,
                    "idn": idn_np,
                }
            )
    return in_maps


def kernel(x, w_qkv, b_qkv, w_proj, b_proj, _trace=False, _trace_kwargs=None):
    x = np.asarray(x, dtype=np.float32)
    w_qkv = np.asarray(w_qkv, dtype=np.float32)
    b_qkv = np.asarray(b_qkv, dtype=np.float32)
    w_proj = np.asarray(w_proj, dtype=np.float32)
    b_proj = np.asarray(b_proj, dtype=np.float32)

    nc = _get_nc()
    in_maps = _shard_inputs(x, w_qkv, b_qkv, w_proj)
    res = run_bass_kernel_spmd(
        nc, in_maps, core_ids=list(range(8)), trace=_trace,
        **(_trace_kwargs or {}),
    )
    out = np.empty((B, T, C), np.float32)
    for b in range(B):
        out[b] = res.results[2 * b]["out"] + res.results[2 * b + 1]["out"] + b_proj
    if _trace:
        return out, res
    return out


# revision 11
# speedup vs baseline: 1.1207x; 1.0491x over previous
"""Causal self-attention (B=4, T=2048, C=1024, H=16) on 8 TRN2 NeuronCores.

Sharding: 8 cores = 4 batches x 2 head-groups (Megatron tensor-parallel over
heads + data-parallel over batch). Each core computes, for its batch b and its
8 heads, qkv projection -> causal attention -> its partial output projection.
Host sums the two partial outputs per batch and adds b_proj.

Schedule: one fused instruction stream. The attention jt-units (QK matmul ->
exp -> PV matmul) are inherently Scalar-engine(exp)-paced: ~1.03us of ACT work
per 512-col j-tile vs ~850ns of PE work. So independent "filler" matmul groups
(next band's qkv projection, previous bands' output projection) are sprinkled
INTO the jt stream at an even pace, sized per band to cover its exp deficit:
  band 0: s1(1)            band 1: s1(2)+proj(0)
  band 2: s1(3)            band 3: proj(1)+proj(2)
Weight loads ride the second (Activation) HWDGE queue in parallel with x loads
on the SP queue; x is prefetched two bands ahead.

dtypes: q/k/v/att/y bf16 (moving-operand bf16 keeps every attention matmul at
1 cycle/row; fp32r drops to 4 cyc/row below 256 free columns), x/wqk/wv f32r,
wp bf16 (matmul dtype pairing with bf16 y), all accumulation f32 in PSUM.
exp needs no max-subtraction: logits are O(1) by construction, 1/sqrt(hd)
folded into w_q on the host. Causality at 128-column granularity; the in-tile
triangle is applied by accumulating a -1e5 triangular bf16 mask into PSUM via
an identity matmul. Softmax sums come free from an appended ones-column on V;
normalization is reciprocal + partition_broadcast on the unnormalized PV
output (the e=1 head needs a 64->127 partition-shift DMA, DVE is lane-locked).
"""

import sys

if "/opt/trn_rl_repo" not in sys.path:
    sys.path.insert(0, "/opt/trn_rl_repo")

from contextlib import ExitStack

import numpy as np
import ml_dtypes

import concourse.bass as bass
import concourse.tile as tile
from concourse import bacc, mybir
from concourse.bass_utils import run_bass_kernel_spmd

F32 = mybir.dt.float32
F32R = mybir.dt.float32r
BF16 = mybir.dt.bfloat16
AF = mybir.ActivationFunctionType

B, T, C = 4, 2048, 1024
H, HD = 16, 64
NHL = 8          # heads per core (local)
NPAIR = 4        # head pairs per core
P = 128
TQ = 512         # query tile (free dim)
TJ = 128         # key tile (partitions)
NIT = T // TQ    # 4 query tile bands
NTS = T // P     # 16 token sub-tiles
NCT = C // P     # 8 contraction tiles over C
NEG = -100000.0  # additive causal mask value


def build_kernel(trace_label=None):
    nc = bacc.Bacc("TRN2", target_bir_lowering=False)

    xt = nc.declare_dram_parameter("xt", [C, T], F32R, isOutput=False)
    wqk = nc.declare_dram_parameter("wqk", [P, 8, 1024], F32R, isOutput=False)
    bqk = nc.declare_dram_parameter("bqk", [P, 8], F32, isOutput=False)
    wv = nc.declare_dram_parameter("wv", [P, NCT, 512], F32R, isOutput=False)
    bv = nc.declare_dram_parameter("bv", [1, 512], F32, isOutput=False)
    wp = nc.declare_dram_parameter("wp", [P, NPAIR, 1024], BF16, isOutput=False)
    tri = nc.declare_dram_parameter("tri", [P, P], BF16, isOutput=False)
    idn = nc.declare_dram_parameter("idn", [P, P], BF16, isOutput=False)
    out = nc.declare_dram_parameter("out", [T, C], F32, isOutput=True)

    with tile.TileContext(nc) as tc, ExitStack() as ctx:
        persist = ctx.enter_context(tc.tile_pool(name="persist", bufs=1))

        q_sb = persist.tile([P, NPAIR, T], BF16)   # queries; later y^T (reuse)
        k_sb = persist.tile([P, NPAIR, T], BF16)
        v_sb = persist.tile([P, NTS, NHL, HD + 1], BF16)
        bqk_sb = persist.tile([P, 8], F32)
        bv_sb = persist.tile([P, 512], F32)
        tri_sb = persist.tile([P, P], BF16)
        idn_sb = persist.tile([P, P], BF16)
        wqk_sb = persist.tile([P, 8, 1024], F32R)
        wv_sb = persist.tile([P, NCT, 512], F32R)
        wp_sb = persist.tile([P, NPAIR, 1024], BF16)

        s1x = ctx.enter_context(tc.tile_pool(name="s1x", bufs=16))
        attp = ctx.enter_context(tc.tile_pool(name="attp", bufs=6))
        nrm = ctx.enter_context(tc.tile_pool(name="nrm", bufs=2))
        s3o = ctx.enter_context(tc.tile_pool(name="s3o", bufs=4))
        mmps = ctx.enter_context(tc.tile_pool(name="mmps", bufs=2, space="PSUM"))
        qkps = ctx.enter_context(tc.tile_pool(name="qkps", bufs=2, space="PSUM"))
        pvps = ctx.enter_context(tc.tile_pool(name="pvps", bufs=2, space="PSUM"))

        # setup constants on the Activation HWDGE queue; x(0) is split across
        # both queues so the first matmul group can start ~3us earlier
        nc.scalar.dma_start(bqk_sb, bqk[:])
        nc.scalar.dma_start(tri_sb, tri[:])
        nc.scalar.dma_start(idn_sb, idn[:])
        nc.scalar.dma_start(bv_sb[0:1, :], bv[:])
        nc.gpsimd.partition_broadcast(bv_sb[:, :], bv_sb[0:1, :])
        nc.vector.memset(v_sb[:, :, :, HD : HD + 1], 1.0)

        xc_tiles = {}

        def load_x(t, split=False):
            for c in range(NCT):
                xi = s1x.tile([P, TQ], F32R, tag="xc", name=f"x_{t}_{c}")
                eng = nc.scalar if (split and c % 2) else nc.sync
                eng.dma_start(xi, xt[c * P : (c + 1) * P, t * TQ : (t + 1) * TQ])
                xc_tiles[(t, c)] = xi

        load_x(0, split=True)
        # weight chunks stream after x(0): the t=0 accumulation groups pace
        # with DMA arrival instead of stalling on a monolithic load
        for i in range(NCT):
            nc.scalar.dma_start(wqk_sb[:, i, :], wqk[:, i, :])
        for i in range(NCT):
            nc.scalar.dma_start(wv_sb[:, i, :], wv[:, i, :])
        nc.scalar.dma_start(wp_sb, wp[:])
        load_x(1)

        def emit_s1_qk(t, m):
            # q (m 0-3) / k (m 4-7) feature block: out [f-part, t-free]
            ps = mmps.tile([P, TQ], F32, tag="mm", name=f"s1qk_{t}_{m}")
            for c in range(NCT):
                nc.tensor.matmul(
                    ps,
                    wqk_sb[:, m, c * P : (c + 1) * P],
                    xc_tiles[(t, c)],
                    start=(c == 0),
                    stop=(c == NCT - 1),
                )
            dst = q_sb if m < 4 else k_sb
            nc.vector.tensor_scalar_add(
                dst[:, m % 4, t * TQ : (t + 1) * TQ], ps, bqk_sb[:, m : m + 1]
            )

        def emit_s1_v(t, s):
            # v block: out [t-part, f-free(head-major)] + bias, ones col kept
            ps = mmps.tile([P, 512], F32, tag="mm", name=f"s1v_{t}_{s}")
            for c in range(NCT):
                nc.tensor.matmul(
                    ps,
                    xc_tiles[(t, c)][:, s * P : (s + 1) * P],
                    wv_sb[:, c, :],
                    start=(c == 0),
                    stop=(c == NCT - 1),
                )
            nc.vector.tensor_tensor(
                v_sb[:, t * 4 + s, :, 0:HD],
                ps.rearrange("p (h d) -> p h d", h=NHL),
                bv_sb.rearrange("p (h d) -> p h d", h=NHL),
                mybir.AluOpType.add,
            )

        proj_n = [0]

        def emit_proj(tt, ot, tail=False):
            # in the tail (attention done) alternate psum between the mm and
            # the freed pv pools: 4-deep rotation hides the copy-sem latency
            if tail and proj_n[0] % 2:
                ps = pvps.tile([P, 512], F32, tag="pv", name=f"s3_{tt}_{ot}")
            else:
                ps = mmps.tile([P, 512], F32, tag="mm", name=f"s3_{tt}_{ot}")
            for a in range(NPAIR):
                nc.tensor.matmul(
                    ps,
                    q_sb[:, a, tt * P : (tt + 1) * P],
                    wp_sb[:, a, ot * 512 : (ot + 1) * 512],
                    start=(a == 0),
                    stop=(a == NPAIR - 1),
                )
            ot_sb = s3o.tile([P, 512], F32, tag="osb", name=f"o_{tt}_{ot}")
            nc.vector.tensor_copy(ot_sb, ps)
            eng = nc.scalar if proj_n[0] % 2 else nc.sync
            proj_n[0] += 1
            eng.dma_start(out[tt * P : (tt + 1) * P, ot * 512 : (ot + 1) * 512], ot_sb)

        def normalize(a, it, pv):
            i0 = it * TQ
            # e=1 first: its chain is longer (partition-shift DMA at the end).
            # partition_broadcast only reads partition 0, so the reciprocal
            # (lane-locked to the sums row 64) hops 64->0 via DMA; the two
            # chains ride different HWDGE queues.
            for e in (1, 0):
                rb = nrm.tile([P, TQ], F32, tag="rb", name=f"rb_{a}_{it}_{e}")
                nc.vector.reciprocal(rb[HD : HD + 1, :], pv[e][HD : HD + 1, :])
                eng = nc.sync if e else nc.scalar
                eng.dma_start(rb[0:1, :], rb[HD : HD + 1, :])
                nc.gpsimd.partition_broadcast(rb[0:HD, :], rb[0:1, :])
                if e == 0:
                    nc.vector.tensor_mul(
                        q_sb[0:HD, a, i0 : i0 + TQ], pv[e][0:HD, :], rb[0:HD, :]
                    )
                else:
                    yt = nrm.tile([P, TQ], BF16, tag="yt", name=f"yt_{a}_{it}")
                    nc.vector.tensor_mul(yt[0:HD, :], pv[e][0:HD, :], rb[0:HD, :])
                    nc.scalar.dma_start(q_sb[64:128, a, i0 : i0 + TQ], yt[0:HD, :])

        def emit_attn_band(it, extra):
            i0 = it * TQ
            njt = (i0 + TQ) // TJ
            total_jt = njt * NPAIR
            nfill = len(extra)
            filled = 0
            jcount = 0
            for a in range(NPAIR):
                pv = [
                    pvps.tile([P, TQ], F32, tag="pv", name=f"pv{e}_{a}_{it}")
                    for e in (0, 1)
                ]
                prev = None
                for jt in range(njt):
                    j0 = jt * TJ
                    d = j0 - i0
                    istart = max(d, 0)
                    nn = TQ - istart
                    # one 2-bank psum tile holds both heads' S^T blocks;
                    # a single fused exp call halves ACT instruction count
                    qk = qkps.tile([P, 2, TQ], F32, tag="qk", name=f"qk_{a}_{it}_{jt}")
                    for e in (0, 1):
                        nc.tensor.matmul(
                            qk[:, e, istart:TQ],
                            k_sb[64 * e : 64 * e + 64, a, j0 : j0 + TJ],
                            q_sb[64 * e : 64 * e + 64, a, i0 + istart : i0 + TQ],
                            start=True,
                            stop=(d < 0),
                            tile_position=(64 * e, 0),
                        )
                        if d >= 0:
                            nc.tensor.matmul(
                                qk[:, e, istart : istart + TJ],
                                idn_sb,
                                tri_sb,
                                start=False,
                                stop=True,
                                tile_position=(0, 0),
                            )
                    att = attp.tile(
                        [P, 2, TQ], BF16, tag="att", name=f"att_{a}_{it}_{jt}"
                    )
                    nc.scalar.activation(att[:, :, 0:nn], qk[:, :, istart:TQ], AF.Exp)
                    if prev is not None:
                        pjt, patt, pnn, pistart = prev
                        for e in (0, 1):
                            nc.tensor.matmul(
                                pv[e][0 : HD + 1, pistart:TQ],
                                v_sb[:, pjt, 2 * a + e, :],
                                patt[:, e, 0:pnn],
                                start=(pjt == 0),
                                stop=(pjt == njt - 1),
                            )
                    prev = (jt, att, nn, istart)
                    jcount += 1
                    # even fractional pacing of fillers over the jt stream
                    while filled < nfill and filled < (jcount * nfill) // total_jt:
                        extra[filled]()
                        filled += 1
                pjt, patt, pnn, pistart = prev
                for e in (0, 1):
                    nc.tensor.matmul(
                        pv[e][0 : HD + 1, pistart:TQ],
                        v_sb[:, pjt, 2 * a + e, :],
                        patt[:, e, 0:pnn],
                        start=(pjt == 0),
                        stop=(pjt == njt - 1),
                    )
                normalize(a, it, pv)
            while filled < nfill:
                extra[filled]()
                filled += 1

        def s1_units(t):
            return [lambda m=m: emit_s1_qk(t, m) for m in range(8)] + [
                lambda s=s: emit_s1_v(t, s) for s in range(4)
            ]

        def proj_units(tb):
            return [
                lambda tt=tt, ot=ot: emit_proj(tt, ot)
                for tt in range(tb * 4, tb * 4 + 4)
                for ot in range(2)
            ]

        # band 0 stage 1 runs standalone (nothing to overlap with yet)
        for u in s1_units(0):
            u()

        # filler assignment sized to each band's exp deficit (see module doc)
        fillers = {
            0: s1_units(1),
            1: s1_units(2) + proj_units(0),
            2: s1_units(3),
            3: proj_units(1) + proj_units(2),
        }
        for it in range(NIT):
            if it + 2 < NIT:
                load_x(it + 2)
            emit_attn_band(it, fillers[it])

        for tt in range(12, 16):
            for ot in range(2):
                emit_proj(tt, ot, tail=True)

    nc.compile()
    return nc


_NC_CACHE = None


def _get_nc():
    global _NC_CACHE
    if _NC_CACHE is None:
        _NC_CACHE = build_kernel()
    return _NC_CACHE


def _shard_inputs(x, w_qkv, b_qkv, w_proj):
    """Build the 8 per-core input maps. Core id = 2*batch + head_group."""
    tri_np = np.where(
        np.arange(P)[None, :] >= np.arange(P)[:, None], 0.0, NEG
    ).astype(ml_dtypes.bfloat16)
    idn_np = np.eye(P, dtype=ml_dtypes.bfloat16)

    in_maps = []
    for b in range(B):
        xt = np.ascontiguousarray(x[b].T)  # [C, T]
        for g in range(2):
            s = slice(g * 512, (g + 1) * 512)
            wqk_full = np.concatenate(
                [w_qkv[0:1024][s] / 8.0, w_qkv[1024:2048][s]], axis=0
            )  # [1024 f, 1024 c]
            # m-major chunks: wqk_arr[p, m, c*128+j] = wqk_full[m*128+j, c*128+p]
            wqk_arr = np.ascontiguousarray(
                wqk_full.T.reshape(NCT, P, 8, P).transpose(1, 2, 0, 3).reshape(P, 8, 1024)
            )
            bqk_full = np.concatenate([b_qkv[0:1024][s] / 8.0, b_qkv[1024:2048][s]])
            bqk_arr = np.ascontiguousarray(bqk_full.reshape(8, P).T)
            wv_rows = w_qkv[2048:3072][s]  # [512 f, 1024 c]
            wv_arr = np.ascontiguousarray(
                wv_rows.T.reshape(NCT, P, 512).transpose(1, 0, 2)
            )
            bv_arr = np.ascontiguousarray(b_qkv[2048:3072][s][None, :])
            wp_rhs = w_proj[:, s].T  # [512 hd, 1024 o]
            wp_arr = np.ascontiguousarray(
                wp_rhs.reshape(NPAIR, P, 1024).transpose(1, 0, 2)
            )
            in_maps.append(
                {
                    "xt": xt,
                    "wqk": wqk_arr.astype(np.float32),
                    "bqk": bqk_arr.astype(np.float32),
                    "wv": wv_arr.astype(np.float32),
                    "bv": bv_arr.astype(np.float32),
                    "wp": wp_arr.astype(ml_dtypes.bfloat16),
                    "tri": tri_np,
                    "idn": idn_np,
                }
            )
    return in_maps


def kernel(x, w_qkv, b_qkv, w_proj, b_proj, _trace=False, _trace_kwargs=None):
    x = np.asarray(x, dtype=np.float32)
    w_qkv = np.asarray(w_qkv, dtype=np.float32)
    b_qkv = np.asarray(b_qkv, dtype=np.float32)
    w_proj = np.asarray(w_proj, dtype=np.float32)
    b_proj = np.asarray(b_proj, dtype=np.float32)

    nc = _get_nc()
    in_maps = _shard_inputs(x, w_qkv, b_qkv, w_proj)
    res = run_bass_kernel_spmd(
        nc, in_maps, core_ids=list(range(8)), trace=_trace,
        **(_trace_kwargs or {}),
    )
    out = np.empty((B, T, C), np.float32)
    for b in range(B):
        out[b] = res.results[2 * b]["out"] + res.results[2 * b + 1]["out"] + b_proj
    if _trace:
        return out, res
    return out


# revision 22
# speedup vs baseline: 1.1647x; 1.0393x over previous
"""Causal self-attention (B=4, T=2048, C=1024, H=16) on 8 TRN2 NeuronCores.

Sharding: 8 cores = 4 batches x 2 head-groups (Megatron tensor-parallel over
heads + data-parallel over batch). Each core computes, for its batch b and its
8 heads, qkv projection -> causal attention -> its partial output projection.
Host sums the two partial outputs per batch and adds b_proj.

Schedule: one fused instruction stream. The attention jt-units (QK matmul ->
exp -> PV matmul) are inherently Scalar-engine(exp)-paced: ~1.03us of ACT work
per 512-col j-tile vs ~850ns of PE work. So independent "filler" matmul groups
(next band's qkv projection, previous bands' output projection) are sprinkled
INTO the jt stream at an even pace, sized per band to cover its exp deficit:
  band 0: s1(1)            band 1: s1(2)
  band 2: s1(3)            band 3: proj(0)+proj(1)+proj(2)
Weight loads ride the second (Activation) HWDGE queue in parallel with x loads
on the SP queue; x is prefetched two bands ahead.

dtypes: q/k/v/att/y bf16 (moving-operand bf16 keeps every attention matmul at
1 cycle/row; fp32r drops to 4 cyc/row below 256 free columns), x/wqk/wv f32r,
wp bf16 (matmul dtype pairing with bf16 y), all accumulation f32 in PSUM.
exp needs no max-subtraction: logits are O(1) by construction, 1/sqrt(hd)
folded into w_q on the host. Causality at 128-column granularity; the in-tile
triangle is applied by accumulating a -1e5 triangular bf16 mask into PSUM via
an identity matmul. Softmax sums come free from an appended ones-column on V;
normalization is reciprocal + partition_broadcast on the unnormalized PV
output (the e=1 head needs a 64->127 partition-shift DMA, DVE is lane-locked).
"""

import sys

if "/opt/trn_rl_repo" not in sys.path:
    sys.path.insert(0, "/opt/trn_rl_repo")

from contextlib import ExitStack

import numpy as np
import ml_dtypes

import concourse.bass as bass
import concourse.tile as tile
from concourse import bacc, mybir
from concourse.bass_utils import run_bass_kernel_spmd

F32 = mybir.dt.float32
F32R = mybir.dt.float32r
BF16 = mybir.dt.bfloat16
AF = mybir.ActivationFunctionType

B, T, C = 4, 2048, 1024
H, HD = 16, 64
NHL = 8          # heads per core (local)
NPAIR = 4        # head pairs per core
P = 128
TQ = 512         # query tile (free dim)
TJ = 128         # key tile (partitions)
NIT = T // TQ    # 4 query tile bands
NTS = T // P     # 16 token sub-tiles
NCT = C // P     # 8 contraction tiles over C
NEG = -100000.0  # additive causal mask value


def build_kernel(trace_label=None):
    nc = bacc.Bacc("TRN2", target_bir_lowering=False)

    xt = nc.declare_dram_parameter("xt", [C, T], BF16, isOutput=False)
    wqk = nc.declare_dram_parameter("wqk", [P, 8, 1024], BF16, isOutput=False)
    bqk = nc.declare_dram_parameter("bqk", [P, 8], F32, isOutput=False)
    wv = nc.declare_dram_parameter("wv", [P, NCT, 512], BF16, isOutput=False)
    bv = nc.declare_dram_parameter("bv", [1, 512], F32, isOutput=False)
    wp = nc.declare_dram_parameter("wp", [P, NPAIR, 1024], BF16, isOutput=False)
    tri = nc.declare_dram_parameter("tri", [P, P], BF16, isOutput=False)
    idn = nc.declare_dram_parameter("idn", [P, P], BF16, isOutput=False)
    out = nc.declare_dram_parameter("out", [T, C], F32, isOutput=True)

    with tile.TileContext(nc) as tc, ExitStack() as ctx:
        persist = ctx.enter_context(tc.tile_pool(name="persist", bufs=1))

        q_sb = persist.tile([P, NPAIR, T], BF16)   # queries; later y^T (reuse)
        k_sb = persist.tile([P, NPAIR, T], BF16)
        v_sb = persist.tile([P, NTS, NHL, HD + 1], BF16)
        bqk_sb = persist.tile([P, 8], F32)
        bv_sb = persist.tile([P, 512], F32)
        tri_sb = persist.tile([P, P], BF16)
        idn_sb = persist.tile([P, P], BF16)
        wqk_sb = persist.tile([P, 8, 1024], BF16)
        wv_sb = persist.tile([P, NCT, 512], BF16)
        wp_sb = persist.tile([P, NPAIR, 1024], BF16)

        s1x = ctx.enter_context(tc.tile_pool(name="s1x", bufs=3))
        attp = ctx.enter_context(tc.tile_pool(name="attp", bufs=6))
        nrm = ctx.enter_context(tc.tile_pool(name="nrm", bufs=2))
        s3o = ctx.enter_context(tc.tile_pool(name="s3o", bufs=4))
        mmps = ctx.enter_context(tc.tile_pool(name="mmps", bufs=2, space="PSUM"))
        qkps = ctx.enter_context(tc.tile_pool(name="qkps", bufs=2, space="PSUM"))
        pvps = ctx.enter_context(tc.tile_pool(name="pvps", bufs=2, space="PSUM"))

        # setup constants on the Activation HWDGE queue; x(0) is split across
        # both queues so the first matmul group can start ~3us earlier
        nc.scalar.dma_start(bqk_sb, bqk[:])
        nc.scalar.dma_start(tri_sb, tri[:])
        nc.scalar.dma_start(idn_sb, idn[:])
        nc.scalar.dma_start(bv_sb[0:1, :], bv[:])
        nc.gpsimd.partition_broadcast(bv_sb[:, :], bv_sb[0:1, :])
        nc.vector.memset(v_sb[:, :, :, HD : HD + 1], 1.0)

        xb_tiles = {}

        def load_x(t):
            # one DMA per band: the ~500ns fixed per-DMA cost dominates at
            # bf16 sizes, and consumers need the whole band anyway
            xi = s1x.tile([P, NCT, TQ], BF16, tag="xc", name=f"x_{t}")
            nc.sync.dma_start(
                xi, xt[:, t * TQ : (t + 1) * TQ].rearrange("(c p) t -> p c t", p=P)
            )
            xb_tiles[t] = xi

        load_x(0)
        # wqk streams per m-chunk so the t=0 accumulation groups pace with
        # DMA arrival instead of stalling on a monolithic load
        for i in range(NCT):
            nc.scalar.dma_start(wqk_sb[:, i, :], wqk[:, i, :])
        nc.scalar.dma_start(wv_sb, wv[:])
        nc.scalar.dma_start(wp_sb, wp[:])
        load_x(1)

        def emit_s1_qk(t, m):
            # q (m 0-3) / k (m 4-7) feature block: out [f-part, t-free]
            ps = mmps.tile([P, TQ], F32, tag="mm", name=f"s1qk_{t}_{m}")
            for c in range(NCT):
                nc.tensor.matmul(
                    ps,
                    wqk_sb[:, m, c * P : (c + 1) * P],
                    xb_tiles[t][:, c, :],
                    start=(c == 0),
                    stop=(c == NCT - 1),
                )
            dst = q_sb if m < 4 else k_sb
            nc.vector.tensor_scalar_add(
                dst[:, m % 4, t * TQ : (t + 1) * TQ], ps, bqk_sb[:, m : m + 1]
            )

        def emit_s1_v(t, s):
            # v block: out [t-part, f-free(head-major)] + bias, ones col kept
            ps = mmps.tile([P, 512], F32, tag="mm", name=f"s1v_{t}_{s}")
            for c in range(NCT):
                nc.tensor.matmul(
                    ps,
                    xb_tiles[t][:, c, s * P : (s + 1) * P],
                    wv_sb[:, c, :],
                    start=(c == 0),
                    stop=(c == NCT - 1),
                )
            nc.vector.tensor_tensor(
                v_sb[:, t * 4 + s, :, 0:HD],
                ps.rearrange("p (h d) -> p h d", h=NHL),
                bv_sb.rearrange("p (h d) -> p h d", h=NHL),
                mybir.AluOpType.add,
            )

        proj_n = [0]

        def emit_proj(tt, ot, tail=False):
            # in the tail (attention done) alternate psum between the mm and
            # the freed pv pools: 4-deep rotation hides the copy-sem latency
            if tail and proj_n[0] % 2:
                ps = pvps.tile([P, 512], F32, tag="pv", name=f"s3_{tt}_{ot}")
            else:
                ps = mmps.tile([P, 512], F32, tag="mm", name=f"s3_{tt}_{ot}")
            for a in range(NPAIR):
                nc.tensor.matmul(
                    ps,
                    q_sb[:, a, tt * P : (tt + 1) * P],
                    wp_sb[:, a, ot * 512 : (ot + 1) * 512],
                    start=(a == 0),
                    stop=(a == NPAIR - 1),
                )
            ot_sb = s3o.tile([P, 512], F32, tag="osb", name=f"o_{tt}_{ot}")
            nc.vector.tensor_copy(ot_sb, ps)
            eng = nc.scalar if proj_n[0] % 2 else nc.sync
            proj_n[0] += 1
            eng.dma_start(out[tt * P : (tt + 1) * P, ot * 512 : (ot + 1) * 512], ot_sb)

        def normalize(a, it, pv):
            i0 = it * TQ
            # e=1 first: its chain is longer (partition-shift DMA at the end).
            # partition_broadcast only reads partition 0, so the reciprocal
            # (lane-locked to the sums row 64) hops 64->0 via DMA; the two
            # chains ride different HWDGE queues.
            for e in (1, 0):
                rb = nrm.tile([P, TQ], F32, tag="rb", name=f"rb_{a}_{it}_{e}")
                nc.vector.reciprocal(rb[HD : HD + 1, :], pv[e][HD : HD + 1, :])
                eng = nc.sync if e else nc.scalar
                eng.dma_start(rb[0:1, :], rb[HD : HD + 1, :])
                nc.gpsimd.partition_broadcast(rb[0:HD, :], rb[0:1, :])
                if e == 0:
                    nc.vector.tensor_mul(
                        q_sb[0:HD, a, i0 : i0 + TQ], pv[e][0:HD, :], rb[0:HD, :]
                    )
                else:
                    yt = nrm.tile([P, TQ], BF16, tag="yt", name=f"yt_{a}_{it}")
                    nc.vector.tensor_mul(yt[0:HD, :], pv[e][0:HD, :], rb[0:HD, :])
                    nc.scalar.dma_start(q_sb[64:128, a, i0 : i0 + TQ], yt[0:HD, :])

        def emit_attn_band(it, extra):
            i0 = it * TQ
            njt = (i0 + TQ) // TJ
            total_jt = njt * NPAIR
            nfill = len(extra)
            filled = 0
            jcount = 0
            for a in range(NPAIR):
                pv = [
                    pvps.tile([P, TQ], F32, tag="pv", name=f"pv{e}_{a}_{it}")
                    for e in (0, 1)
                ]
                prev = None
                for jt in range(njt):
                    j0 = jt * TJ
                    d = j0 - i0
                    istart = max(d, 0)
                    nn = TQ - istart
                    # one 2-bank psum tile holds both heads' S^T blocks;
                    # a single fused exp call halves ACT instruction count
                    qk = qkps.tile([P, 2, TQ], F32, tag="qk", name=f"qk_{a}_{it}_{jt}")
                    for e in (0, 1):
                        nc.tensor.matmul(
                            qk[:, e, istart:TQ],
                            k_sb[64 * e : 64 * e + 64, a, j0 : j0 + TJ],
                            q_sb[64 * e : 64 * e + 64, a, i0 + istart : i0 + TQ],
                            start=True,
                            stop=(d < 0),
                            tile_position=(64 * e, 0),
                        )
                        if d >= 0:
                            nc.tensor.matmul(
                                qk[:, e, istart : istart + TJ],
                                idn_sb,
                                tri_sb,
                                start=False,
                                stop=True,
                                tile_position=(0, 0),
                            )
                    att = attp.tile(
                        [P, 2, TQ], BF16, tag="att", name=f"att_{a}_{it}_{jt}"
                    )
                    nc.scalar.activation(att[:, :, 0:nn], qk[:, :, istart:TQ], AF.Exp)
                    if prev is not None:
                        pjt, patt, pnn, pistart = prev
                        for e in (0, 1):
                            nc.tensor.matmul(
                                pv[e][0 : HD + 1, pistart:TQ],
                                v_sb[:, pjt, 2 * a + e, :],
                                patt[:, e, 0:pnn],
                                start=(pjt == 0),
                                stop=(pjt == njt - 1),
                            )
                    prev = (jt, att, nn, istart)
                    jcount += 1
                    # even fractional pacing of fillers over the jt stream
                    while filled < nfill and filled < (jcount * nfill) // total_jt:
                        extra[filled]()
                        filled += 1
                pjt, patt, pnn, pistart = prev
                for e in (0, 1):
                    nc.tensor.matmul(
                        pv[e][0 : HD + 1, pistart:TQ],
                        v_sb[:, pjt, 2 * a + e, :],
                        patt[:, e, 0:pnn],
                        start=(pjt == 0),
                        stop=(pjt == njt - 1),
                    )
                normalize(a, it, pv)
            while filled < nfill:
                extra[filled]()
                filled += 1

        def s1_units(t):
            return [lambda m=m: emit_s1_qk(t, m) for m in range(8)] + [
                lambda s=s: emit_s1_v(t, s) for s in range(4)
            ]

        def proj_units(tb):
            return [
                lambda tt=tt, ot=ot: emit_proj(tt, ot)
                for tt in range(tb * 4, tb * 4 + 4)
                for ot in range(2)
            ]

        # band 0 stage 1 runs standalone (nothing to overlap with yet)
        for u in s1_units(0):
            u()

        # filler assignment sized to each band's exp deficit (see module doc)
        fillers = {
            0: s1_units(1),
            1: s1_units(2),
            2: s1_units(3),
            3: proj_units(0) + proj_units(1) + proj_units(2),
        }
        for it in range(NIT):
            if it + 2 < NIT:
                load_x(it + 2)
            emit_attn_band(it, fillers[it])

        for tt in range(12, 16):
            for ot in range(2):
                emit_proj(tt, ot, tail=True)

    nc.compile()
    return nc


_NC_CACHE = None


def _get_nc():
    global _NC_CACHE
    if _NC_CACHE is None:
        _NC_CACHE = build_kernel()
    return _NC_CACHE


def _shard_inputs(x, w_qkv, b_qkv, w_proj):
    """Build the 8 per-core input maps. Core id = 2*batch + head_group."""
    tri_np = np.where(
        np.arange(P)[None, :] >= np.arange(P)[:, None], 0.0, NEG
    ).astype(ml_dtypes.bfloat16)
    idn_np = np.eye(P, dtype=ml_dtypes.bfloat16)

    in_maps = []
    for b in range(B):
        xt = np.ascontiguousarray(x[b].T).astype(ml_dtypes.bfloat16)  # [C, T]
        for g in range(2):
            s = slice(g * 512, (g + 1) * 512)
            wqk_full = np.concatenate(
                [w_qkv[0:1024][s] / 8.0, w_qkv[1024:2048][s]], axis=0
            )  # [1024 f, 1024 c]
            # m-major chunks: wqk_arr[p, m, c*128+j] = wqk_full[m*128+j, c*128+p]
            wqk_arr = np.ascontiguousarray(
                wqk_full.T.reshape(NCT, P, 8, P).transpose(1, 2, 0, 3).reshape(P, 8, 1024)
            )
            bqk_full = np.concatenate([b_qkv[0:1024][s] / 8.0, b_qkv[1024:2048][s]])
            bqk_arr = np.ascontiguousarray(bqk_full.reshape(8, P).T)
            wv_rows = w_qkv[2048:3072][s]  # [512 f, 1024 c]
            wv_arr = np.ascontiguousarray(
                wv_rows.T.reshape(NCT, P, 512).transpose(1, 0, 2)
            )
            bv_arr = np.ascontiguousarray(b_qkv[2048:3072][s][None, :])
            wp_rhs = w_proj[:, s].T  # [512 hd, 1024 o]
            wp_arr = np.ascontiguousarray(
                wp_rhs.reshape(NPAIR, P, 1024).transpose(1, 0, 2)
            )
            in_maps.append(
                {
                    "xt": xt,
                    "wqk": wqk_arr.astype(ml_dtypes.bfloat16),
                    "bqk": bqk_arr.astype(np.float32),
                    "wv": wv_arr.astype(ml_dtypes.bfloat16),
                    "bv": bv_arr.astype(np.float32),
                    "wp": wp_arr.astype(ml_dtypes.bfloat16),
                    "tri": tri_np,
                    "idn": idn_np,
                }
            )
    return in_maps


def kernel(x, w_qkv, b_qkv, w_proj, b_proj, _trace=False, _trace_kwargs=None):
    x = np.asarray(x, dtype=np.float32)
    w_qkv = np.asarray(w_qkv, dtype=np.float32)
    b_qkv = np.asarray(b_qkv, dtype=np.float32)
    w_proj = np.asarray(w_proj, dtype=np.float32)
    b_proj = np.asarray(b_proj, dtype=np.float32)

    nc = _get_nc()
    in_maps = _shard_inputs(x, w_qkv, b_qkv, w_proj)
    res = run_bass_kernel_spmd(
        nc, in_maps, core_ids=list(range(8)), trace=_trace,
        **(_trace_kwargs or {}),
    )
    out = np.empty((B, T, C), np.float32)
    for b in range(B):
        out[b] = res.results[2 * b]["out"] + res.results[2 * b + 1]["out"] + b_proj
    if _trace:
        return out, res
    return out
